# revision 18
# baseline (speedup 1.0000x reference)
"""SSIM loss kernel for Trainium2 (8 NeuronCores, data-parallel over batch).

Computes mean(ssim_map(img1, img2)) with an 11x11 Gaussian window (sigma=1.5),
matching the reference jax implementation (NCHW conv, zero "same" padding).

Strategy per core (8 images of 512x512 each):
  - 4 blur fields per image: i1, i2, S = i1^2+i2^2, P = i1*i2
    (only b11+b22 is ever used by SSIM, so the squares share one field).
  - Separable blur = two banded matmul passes on the TensorEngine with the
    512x512 band matrix W (symmetric, 11 diagonals):
      pass A: lhsT = image block (stationary), rhs = W row-block
              -> Y^T = X^T W  (blur along h, output transposed)  [PSUM]
      copy PSUM -> SBUF fp16 (YT)
      pass B: lhsT = W block (stationary), rhs = YT row-block
              -> Z^T = (blur along w of Y)^T                      [PSUM]
  - SSIM map + reduction on DVE/ACT with fused scalar_tensor_tensor ops and
    tensor_tensor_reduce; 1/den via exp(-ln(den)) on ACT (single table set).
  - Per-core output: acc[128, 32] fp32 partial sums; host sums and divides.
"""

import os
import sys

os.environ.setdefault("MYCRO_LOCAL_CACHE", "1")
sys.path.insert(0, "/opt/trn_rl_repo")

import numpy as np

WS = 11
SIGMA = 1.5
C1 = 0.01**2
C2 = 0.03**2
CC = C1 + C2

B, CH, H, W = 64, 1, 512, 512
NCORES = 8
IPC = B // NCORES  # images per core
NB = 4  # 128-row blocks per image dim
PAD = WS // 2

_CACHE = {}


def _window1d() -> np.ndarray:
    xs = np.arange(WS) - WS // 2
    g = np.exp(-(xs.astype(np.float32) ** 2) / (2.0 * SIGMA**2))
    g = g / g.sum()
    return g.astype(np.float32)  # (11,)


def _window1d_f16() -> np.ndarray:
    """f16 taps tweaked so they sum to exactly 1.0 (in exact arithmetic).

    Without this, sum(taps) = 1 + eps and blur(x*y) - blur(x)*blur(y) leaks a
    (s^2 - s^4)*mu_x*mu_y term that systematically biases sigma12 (~8% on the
    final mean). Center tap is re-rounded against the side sum; the dyadic
    residual is absorbed exactly by the outermost (tiny) taps.
    """
    g = _window1d().astype(np.float64)
    gt = g.astype(np.float16).astype(np.float64).copy()
    T = gt[PAD + 1 :].sum()  # right-side taps (exact dyadic sum)
    g0 = np.float64(np.float16(1.0 - 2.0 * T))
    resid = 1.0 - g0 - 2.0 * T
    edge = gt[-1] + resid / 2.0
    assert np.float64(np.float16(edge)) == edge, "residual not f16-representable"
    gt[PAD] = g0
    gt[0] = edge
    gt[-1] = edge
    assert gt.sum() == 1.0
    return gt.astype(np.float16)


def _band_matrix(g=None) -> np.ndarray:
    """Wb[r, c] = g[c - r + 5] for |c - r| <= 5 else 0  (512x512, symmetric)."""
    if g is None:
        g = _window1d()
    Wb = np.zeros((H, H), dtype=np.float32)
    idx = np.arange(H)
    for k in range(-PAD, PAD + 1):
        rows = idx[(idx + k >= 0) & (idx + k < H)]
        Wb[rows, rows + k] = g[k + PAD]
    return Wb


def _block(x: np.ndarray) -> np.ndarray:
    """[..., 512, 512] -> [..., 128, 4, 512] (partition, h-block, w)."""
    lead = x.shape[:-2]
    y = x.reshape(*lead, NB, 128, x.shape[-1])
    y = np.moveaxis(y, -3, -2)  # [..., 128, NB, w]
    return np.ascontiguousarray(y)


def _build_module():
    import concourse.bacc as bacc
    import concourse.tile as tile
    from concourse import mybir

    f16 = mybir.dt.float16
    f32 = mybir.dt.float32
    Alu = mybir.AluOpType
    Act = mybir.ActivationFunctionType

    nc = bacc.Bacc(
        "TRN2",
        target_bir_lowering=False,
        debug=False,
    )

    imgs_d = nc.dram_tensor("imgs", [IPC, 2, 128, NB, W], f16, kind="ExternalInput")
    wb_d = nc.dram_tensor("wband", [128, NB, H], f16, kind="ExternalInput")
    wb2_d = nc.dram_tensor("wband2", [128, NB, H], f16, kind="ExternalInput")
    acc_d = nc.dram_tensor("acc", [128, IPC * NB], f32, kind="ExternalOutput")

    # band column ranges for pass A (j>0); j==0 streams the full width so the
    # accumulation group covers the whole psum region with start=True.
    def acols(j):
        lo = 128 * j - PAD
        hi = min(H, 128 * j + 128 + PAD)
        return lo, hi

    with tile.TileContext(nc) as tc:
        with (
            tc.tile_pool(name="consts", bufs=1) as consts,
            tc.tile_pool(name="imgin", bufs=2) as imgin,
            tc.tile_pool(name="prod", bufs=2) as prod,
            tc.tile_pool(name="yt", bufs=2) as ytp,
            tc.tile_pool(name="psA", bufs=2, space="PSUM") as psA,
            tc.tile_pool(name="psB", bufs=1, space="PSUM") as psB,
            tc.tile_pool(name="mapt", bufs=3) as mapt,
            tc.tile_pool(name="accp", bufs=1) as accp,
        ):
            wsb = consts.tile([128, NB, H], f16, tag="wsb")
            nc.sync.dma_start(out=wsb, in_=wb_d.ap())
            wsb2 = consts.tile([128, NB, H], f16, tag="wsb2")
            nc.sync.dma_start(out=wsb2, in_=wb2_d.ap())

            acc = accp.tile([128, IPC * NB], f32, tag="acc")

            for m in range(IPC):
                ii = imgin.tile([128, 2, NB, W], f16, tag="ii")
                nc.sync.dma_start(
                    out=ii, in_=imgs_d.ap()[m].rearrange("c p b w -> p c b w")
                )
                i1b = ii[:, 0]
                i2b = ii[:, 1]

                # stage 1: products (scalar_tensor_tensor: the plain TT ISA
                # struct only fits one sync wait command)
                sq1 = prod.tile([128, NB, W], f16, tag="sq1")
                sq2 = prod.tile([128, NB, W], f16, tag="sq2")
                ps = prod.tile([128, NB, W], f16, tag="ps")
                p12 = prod.tile([128, NB, W], f16, tag="p12")
                nc.scalar.square(out=sq1, in_=i1b)
                nc.scalar.square(out=sq2, in_=i2b)
                nc.vector.scalar_tensor_tensor(
                    ps, sq1, 1.0, sq2, op0=Alu.mult, op1=Alu.add
                )
                nc.vector.scalar_tensor_tensor(
                    p12, i1b, 1.0, i2b, op0=Alu.mult, op1=Alu.mult
                )

                fields = [i1b, i2b, ps, p12]
                yts = [
                    ytp.tile([128, NB, H], f16, tag=f"yt{f}", name=f"yt{f}")
                    for f in range(4)
                ]

                # pass A: Y^T = X^T W per field (blur along h, transposed out)
                # field 3 (i1*i2) uses 2W so its final pass-B output is 2*b12.
                for f in range(4):
                    src = fields[f]
                    wa = wsb2 if f == 3 else wsb
                    for half in range(2):  # wblocks (2*half, 2*half+1)
                        ya = psA.tile([128, 2, H], f32, tag="ya")
                        for kk in range(2):
                            wbk = 2 * half + kk
                            # lhsT free dims: [NB, 128] -> take per-j slices
                            for j in range(NB):
                                lj = src[:, j, 128 * wbk : 128 * wbk + 128]
                                if j == 0:
                                    nc.tensor.matmul(
                                        ya[:, kk, :],
                                        lj,
                                        wa[:, 0, :],
                                        start=True,
                                        stop=False,
                                    )
                                else:
                                    lo, hi = acols(j)
                                    nc.tensor.matmul(
                                        ya[:, kk, lo:hi],
                                        lj,
                                        wa[:, j, lo:hi],
                                        start=False,
                                        stop=(j == NB - 1),
                                    )
                        # copy PSUM -> SBUF fp16 (split between ACT and DVE)
                        dst = yts[f][:, 2 * half : 2 * half + 2, :]
                        if f < 2:
                            nc.scalar.copy(out=dst, in_=ya)
                        else:
                            nc.vector.tensor_copy(out=dst, in_=ya)

                # pass B + map, per output w'-block i
                for i in range(NB):
                    pm1 = psB.tile([128, H], f32, tag="pm1")
                    pm2 = psB.tile([128, H], f32, tag="pm2")
                    pS = psB.tile([128, H], f32, tag="pS")
                    pP = psB.tile([128, H], f32, tag="pP")
                    outs = [pm1, pm2, pS, pP]
                    js = [j for j in (i - 1, i, i + 1) if 0 <= j < NB]
                    for f in range(4):
                        for jidx, j in enumerate(js):
                            nc.tensor.matmul(
                                outs[f],
                                wsb[:, j, 128 * i : 128 * i + 128],
                                yts[f][:, j, :],
                                start=(jidx == 0),
                                stop=(jidx == len(js) - 1),
                            )

                    col = m * NB + i
                    m2s = mapt.tile([128, H], f16, tag="m2s")
                    t1 = mapt.tile([128, H], f16, tag="t1")
                    t2 = mapt.tile([128, H], f16, tag="t2")
                    t3 = mapt.tile([128, H], f16, tag="t3")
                    a = mapt.tile([128, H], f16, tag="a")
                    ns = mapt.tile([128, H], f16, tag="ns")
                    num = mapt.tile([128, H], f16, tag="num")
                    mc1 = mapt.tile([128, H], f16, tag="mc1")
                    draw = mapt.tile([128, H], f16, tag="draw")
                    den = mapt.tile([128, H], f16, tag="den")
                    lden = mapt.tile([128, H], f16, tag="lden")
                    r16 = mapt.tile([128, H], f16, tag="r16")
                    mp = mapt.tile([128, H], f16, tag="mp")

                    # ACT: copy m2 out of PSUM; square m1 from PSUM
                    nc.scalar.copy(out=m2s, in_=pm2)
                    nc.scalar.square(out=t2, in_=pm1)
                    # DVE (scalar_tensor_tensor instead of tensor_tensor:
                    # the TT ISA struct only fits one sync wait command)
                    nc.vector.scalar_tensor_tensor(
                        t1, pm1, 1.0, m2s, op0=Alu.mult, op1=Alu.mult
                    )  # mu1*mu2
                    nc.vector.scalar_tensor_tensor(
                        t3, m2s, 1.0, m2s, op0=Alu.mult, op1=Alu.mult
                    )  # mu2^2
                    nc.vector.tensor_scalar(
                        a, t1, 2.0, C1, op0=Alu.mult, op1=Alu.add
                    )  # A = 2 t1 + C1
                    nc.vector.scalar_tensor_tensor(
                        ns, t1, -2.0, pP, op0=Alu.mult, op1=Alu.add
                    )  # 2 sig12  (pP = 2*b12)
                    nc.vector.scalar_tensor_tensor(
                        num, ns, C2, a, op0=Alu.add, op1=Alu.mult
                    )  # (2 sig12 + C2) * A
                    nc.vector.scalar_tensor_tensor(
                        mc1, t2, C1, t3, op0=Alu.add, op1=Alu.add
                    )  # msum + C1
                    nc.vector.scalar_tensor_tensor(
                        draw, mc1, -1.0, pS, op0=Alu.mult, op1=Alu.add
                    )  # bS - msum - C1
                    nc.vector.scalar_tensor_tensor(
                        den, draw, CC, mc1, op0=Alu.add, op1=Alu.mult
                    )  # (sig_sum + C2) * (msum + C1)
                    # 1/den via exp(-ln(den)) on ACT (same table set as copy/square)
                    nc.scalar.activation(out=lden, in_=den, func=Act.Ln)
                    nc.scalar.activation(out=r16, in_=lden, func=Act.Exp, scale=-1.0)
                    # (tensor_tensor_reduce faults on this runtime; use
                    # STT multiply + tensor_reduce instead)
                    nc.vector.scalar_tensor_tensor(
                        mp, num, 1.0, r16, op0=Alu.mult, op1=Alu.mult
                    )
                    nc.vector.tensor_reduce(
                        acc[:, col : col + 1],
                        mp,
                        axis=mybir.AxisListType.X,
                        op=Alu.add,
                    )

            nc.sync.dma_start(out=acc_d.ap(), in_=acc)

    nc.compile()
    return nc


def _get_module():
    if "nc" not in _CACHE:
        _CACHE["nc"] = _build_module()
    return _CACHE["nc"]


def kernel(img1: np.ndarray, img2: np.ndarray) -> np.ndarray:
    from concourse.bass_utils import run_bass_kernel_spmd

    nc = _get_module()

    g16 = _window1d_f16()
    wb16 = _block(_band_matrix(g16.astype(np.float32)).astype(np.float16))
    wb16x2 = (wb16.astype(np.float32) * 2.0).astype(np.float16)
    x1 = _block(np.asarray(img1, dtype=np.float32).reshape(B, H, W).astype(np.float16))
    x2 = _block(np.asarray(img2, dtype=np.float32).reshape(B, H, W).astype(np.float16))
    xs = np.ascontiguousarray(np.stack([x1, x2], axis=1))  # [B, 2, 128, NB, W]
    # shard over batch
    in_maps = []
    for c in range(NCORES):
        sl = slice(c * IPC, (c + 1) * IPC)
        in_maps.append({"imgs": xs[sl], "wband": wb16, "wband2": wb16x2})

    res = run_bass_kernel_spmd(nc, in_maps, core_ids=list(range(NCORES)))
    total = 0.0
    for c in range(NCORES):
        total += np.asarray(res.results[c]["acc"], dtype=np.float64).sum()
    return np.float32(total / (B * CH * H * W))
